# revision 40
# baseline (speedup 1.0000x reference)
"""Trainium2 Bass kernel for multi-head causal self-attention.

Tensor-parallel over 8 NeuronCores: each core owns 2 of the 16 heads.
Per core (SPMD, identical program, different weight shards):
  - QKV projections for its 2 heads (bf16 data, f32 psum accumulate)
  - causal attention for its 2 heads (scores kept transposed [k, q];
    softmax denominator via a ones-column fused into the PV matmul,
    PV streams N=129 in bf16)
  - output projection partial (f32r), deferred and woven into the next
    phase so the tensor engine never starves behind scalar-engine exps
Phase schedule (tensor-heavy QKV woven into scalar-heavy attention):
  QKV(b0) -> att(b0) + QKV(b1) woven -> att(b1) + outproj(b0) woven
  -> outproj(b1) tail.
Host: shards weights, pre-transposes X to bf16, sums the 8 partials,
adds bo (+ bv@Wo; bk drops out of softmax).

Engine split: tensor = all matmuls; scalar = exp only; vector = psum
copy-outs, bias adds, reciprocals, mask multiplies, ones memset.
"""
import numpy as np
from contextlib import ExitStack

import ml_dtypes
import concourse.bass as bass
import concourse.tile as tile
from concourse import bacc, mybir
from concourse.bass_utils import run_bass_kernel_spmd

# Problem shape (hardcoded per contract)
B, S, D = 2, 2048, 2048
H, DH = 16, 128
N_CORES = 8
HL = H // N_CORES          # heads per core = 2
DHL = HL * DH              # 256
SC = 512                   # s-chunk for projections
NSC = S // SC              # 8 chunks per batch
NKB = S // 128             # 16 key blocks per batch
NQI = S // 512             # 4 q-chunks of 512 per batch
NDC = D // 128             # 16 contraction blocks

F32 = mybir.dt.float32
F32R = mybir.dt.float32r
BF16 = mybir.dt.bfloat16
AF = mybir.ActivationFunctionType
BF16NP = ml_dtypes.bfloat16

_cached_nc = None


def _mm(nc, out, lhsT, rhs, start, stop):
    nc.tensor.matmul(out, lhsT, rhs, start=start, stop=stop)


def _drain(gen):
    if gen is not None:
        for _ in gen:
            pass


class _MultiPacer:
    """Round-robin several work generators, paced so the backlog drains
    roughly uniformly over the remaining slots."""

    def __init__(self, total_slots):
        self.total_slots = total_slots
        self.slot = 0
        self.gens = []          # (gen, remaining_estimate)
        self.backlog = 0
        self.credit = 0.0
        self.deferred = []

    def add(self, gen, est):
        self.gens.append(gen)
        self.backlog += est

    def defer(self, b, qi):
        self.deferred.append((b, qi))

    def idle(self):
        return not self.gens

    def step(self):
        self.slot += 1
        remaining_slots = max(1, self.total_slots - self.slot)
        if self.gens:
            self.credit += self.backlog / remaining_slots
        k = int(self.credit)
        for _ in range(k):
            if not self.gens:
                break
            gen = self.gens.pop(0)
            try:
                next(gen)
                self.credit -= 1
                self.backlog = max(0, self.backlog - 1)
                self.gens.append(gen)
            except StopIteration:
                pass

    def drain_all(self):
        for gen in self.gens:
            _drain(gen)
        self.gens = []


class _Pacer:
    """Spread `total` weave quanta over `slots` call sites."""

    def __init__(self, gen, total, slots):
        self.gen = gen
        self.total = total
        self.slots = max(slots, 1)
        self.emitted = 0
        self.slot = 0

    def step(self):
        self.slot += 1
        while (self.gen is not None
               and self.emitted * self.slots < self.slot * self.total):
            try:
                next(self.gen)
                self.emitted += 1
            except StopIteration:
                self.gen = None

    def drain(self):
        _drain(self.gen)
        self.gen = None


def build_nc():
    nc = bacc.Bacc("TRN2", target_bir_lowering=False, debug=False, num_devices=N_CORES)

    # X pre-chunked host-side: xt[p, b, sc, n, s'] = X[b, sc*SC+s', n*128+p]
    xt = nc.dram_tensor("xt", [128, B * NSC, NDC, SC], BF16,
                        kind="ExternalInput").ap()
    # weights pre-arranged host-side to partition-major SBUF layout
    wq = nc.dram_tensor("wq", [128, HL, NDC, 128], BF16,
                        kind="ExternalInput").ap()
    wk = nc.dram_tensor("wk", [128, NDC, DHL], BF16, kind="ExternalInput").ap()
    wv = nc.dram_tensor("wv", [128, NDC, DHL], BF16, kind="ExternalInput").ap()
    bqt_d = nc.dram_tensor("bqt", [128, HL], F32, kind="ExternalInput").ap()
    wo = nc.dram_tensor("wo", [HL, 128, D], F32R, kind="ExternalInput").ap()
    mask_d = nc.dram_tensor("maskt", [128, 128], BF16, kind="ExternalInput").ap()
    ident_d = nc.dram_tensor("ident", [128, 128], F32R, kind="ExternalInput").ap()
    identb_d = nc.dram_tensor("identb", [128, 128], BF16, kind="ExternalInput").ap()
    out = nc.dram_tensor("out", [B, S, D], BF16, kind="ExternalOutput").ap()


    with tile.TileContext(nc) as tc, ExitStack() as ctx:
        pp = ctx.enter_context(tc.tile_pool(name="persist", bufs=1))

        wq_t = pp.tile([128, HL, NDC, 128], BF16)
        wk_t = pp.tile([128, NDC, DHL], BF16)
        wv_t = pp.tile([128, NDC, DHL], BF16)
        wo_t = pp.tile([128, HL, D], F32R)
        bqt = pp.tile([128, HL], F32)
        mask = pp.tile([128, 128], BF16)   # additive: 0 on causal, -1e9 above
        ident = pp.tile([128, 128], F32R)
        identb = pp.tile([128, 128], BF16)

        # h0 slice first: the first Q matmul group gates on 0.5 MB, not 1 MB
        nc.sync.dma_start(out=wq_t[:, 0], in_=wq[:, 0])
        nc.sync.dma_start(out=bqt, in_=bqt_d)
        nc.sync.dma_start(out=wq_t[:, 1], in_=wq[:, 1])

        # per-batch persistent activations
        qt = [pp.tile([128, HL, S], BF16, name=f"qt{b}") for b in range(B)]
        kt = [pp.tile([128, HL, S], BF16, name=f"kt{b}") for b in range(B)]
        vcat = [pp.tile([128, NKB, HL, 130], BF16, name=f"vcat{b}")
                for b in range(B)]
        # attn^T blocks awaiting (deferred) output projection
        stb = [pp.tile([128, NQI, 4, HL, 128], F32R, name=f"stb{b}")
               for b in range(B)]

        xp = ctx.enter_context(tc.tile_pool(name="xtp", bufs=3))
        sm = ctx.enter_context(tc.tile_pool(name="smp", bufs=2))
        xpool = ctx.enter_context(tc.tile_pool(name="expool", bufs=2))

        def qkv_quanta(b):
            """Generator: QKV projections for batch b, yielding after each
            16-matmul accumulation group (the weave quantum)."""
            with tc.tile_pool(name=f"psp{b}", bufs=1, space="PSUM") as psp:
                nc.vector.memset(vcat[b][:, :, :, 128:129], 1.0)
                def q_group(sc, xt_t, pqk, slot, h):
                    for dc in range(NDC):
                        _mm(nc, pqk[:, slot, :], wq_t[:, h, dc, :],
                            xt_t[:, dc, :], dc == 0, dc == NDC - 1)
                    nc.vector.tensor_scalar_add(
                        out=qt[b][:, h, sc * SC:(sc + 1) * SC],
                        in0=pqk[:, slot, :], scalar1=bqt[:, h:h + 1])

                def k_group(sc, xt_t, pqk, slot, h):
                    for dc in range(NDC):
                        _mm(nc, pqk[:, slot, :],
                            wk_t[:, dc, h * 128:(h + 1) * 128],
                            xt_t[:, dc, :], dc == 0, dc == NDC - 1)
                    nc.vector.tensor_copy(
                        kt[b][:, h, sc * SC:(sc + 1) * SC], pqk[:, slot, :])

                def v_group(sc, xt_t, pqk, sb):
                    kb = sc * (SC // 128) + sb
                    pv = pqk[:, sb % 2, (sb // 2) * 256:(sb // 2) * 256 + 256]
                    for dc in range(NDC):
                        _mm(nc, pv, xt_t[:, dc, sb * 128:(sb + 1) * 128],
                            wv_t[:, dc, :], dc == 0, dc == NDC - 1)
                    nc.vector.tensor_copy(
                        vcat[b][:, kb, 0, 0:128], pv[:, 0:128])
                    nc.vector.tensor_copy(
                        vcat[b][:, kb, 1, 0:128], pv[:, 128:256])

                for sc in range(NSC):
                    xt_t = xp.tile([128, NDC, SC], BF16, tag="xt")
                    if b == 0 and sc == 0:
                        # two halves so the first matmuls gate on 0.5 MB
                        nc.gpsimd.dma_start(out=xt_t[:, 0:8], in_=xt[:, 0, 0:8])
                        nc.gpsimd.dma_start(out=xt_t[:, 8:16], in_=xt[:, 0, 8:16])
                    else:
                        nc.gpsimd.dma_start(out=xt_t, in_=xt[:, b * NSC + sc])
                    if b == 0 and sc == 0:
                        nc.sync.dma_start(out=wk_t, in_=wk)
                        nc.sync.dma_start(out=wv_t, in_=wv)
                    # q/k/v accumulation groups are strictly sequential,
                    # so one 2-bank tile serves the whole chunk
                    pqk = psp.tile([128, 2, SC], F32, tag="pqk", bufs=1)
                    for h in range(HL):
                        q_group(sc, xt_t, pqk, 0, h)
                        yield
                        k_group(sc, xt_t, pqk, 1, h)
                        yield
                    for sb in range(SC // 128):
                        v_group(sc, xt_t, pqk, sb)
                        yield
                if b == 0:
                    nc.sync.dma_start(out=wo_t[:, 0, :], in_=wo[0])
                    nc.sync.dma_start(out=wo_t[:, 1, :], in_=wo[1])
                    nc.sync.dma_start(out=mask, in_=mask_d)
                    nc.sync.dma_start(out=ident, in_=ident_d)
                    nc.sync.dma_start(out=identb, in_=identb_d)

        def outproj_quanta(b, qi, psa, ctr, qqls=range(4)):
            """Generator: output projection for stb[b][qi], using the po
            tag of the given PSUM pool. Yields per 512-col chunk. Copies
            split 1/3 scalar 2/3 vector: some po-buffer releases bypass
            each queue's head-of-line blocking, without saturating the
            exp-loaded scalar engine."""
            if True:
                for qql in qqls:
                    qq = 4 * qi + qql
                    for dk in range(D // 512):
                        po = psa.tile([128, 512], F32, tag="po", bufs=2)
                        _mm(nc, po, stb[b][:, qi, qql, 0, :],
                            wo_t[:, 0, dk * 512:(dk + 1) * 512], True, False)
                        _mm(nc, po, stb[b][:, qi, qql, 1, :],
                            wo_t[:, 1, dk * 512:(dk + 1) * 512], False, True)
                        ot = sm.tile([128, 512], BF16, tag="ot", bufs=6)
                        ctr[0] += 1
                        if ctr[0] % 3 == 2:
                            nc.scalar.activation(out=ot, in_=po, func=AF.Copy)
                        else:
                            nc.vector.tensor_copy(ot, po)
                        nc.sync.dma_start(
                            out=out[b, qq * 128:(qq + 1) * 128,
                                    dk * 512:(dk + 1) * 512],
                            in_=ot)
                        yield

        def attention_unit(b, qi, psa, weave, op_add=None):
            """One qi-group of causal attention for batch b; calls
            weave.step() at each slot to inject foreign tensor work.
            op_add(qql): called once this qql's attn^T block is complete
            (h==1 done), so its output projection can join the weave."""
            if True:
                for h in range(HL):
                    nkb = 4 * qi + 4
                    pss_t = {}
                    exs = {}

                    def score(kb):
                        dq = max(0, (kb - 4 * qi)) * 128
                        diag = kb >= 4 * qi
                        pss = psa.tile([128, 512], F32, tag="sc", bufs=4)
                        _mm(nc, pss[:, dq:512],
                            kt[b][:, h, kb * 128:(kb + 1) * 128],
                            qt[b][:, h, qi * 512 + dq:(qi + 1) * 512],
                            True, not diag)
                        if diag:
                            # add -1e9 above the diagonal in psum (identity
                            # stationary x additive mask), so exp gives 0 and
                            # no post-exp mask op is needed
                            nc.tensor.matmul(
                                pss[:, dq:dq + 128], identb, mask,
                                start=False, stop=True, skip_group_check=True)
                        pss_t[kb] = pss

                    score(0)
                    for kb in range(nkb):
                        dq = max(0, (kb - 4 * qi)) * 128
                        pss = pss_t.pop(kb)
                        ex = xpool.tile([128, 512], BF16, tag="ex", bufs=18)
                        nc.scalar.activation(
                            out=ex[:, dq:512], in_=pss[:, dq:512],
                            func=AF.Exp)
                        exs[kb] = ex
                        if kb + 1 < nkb:
                            score(kb + 1)
                        weave.step()
                    # qql-outer PV: accumulation groups close sequentially,
                    # so a 2-deep acc ring replaces 4 dedicated banks
                    for qql in range(4):
                        qq = 4 * qi + qql
                        acc = psa.tile([128, 256], F32, tag="acc", bufs=2)
                        for kb in range(qq + 1):
                            _mm(nc, acc[:, 0:129],
                                exs[kb][:, qql * 128:(qql + 1) * 128],
                                vcat[b][:, kb, h, 0:129],
                                kb == 0, kb == qq)
                        rc = sm.tile([128, 1], F32, tag="rc")
                        nc.vector.reciprocal(rc, acc[:, 128:129])
                        an = sm.tile([128, 128], F32R, tag="an")
                        nc.vector.tensor_scalar_mul(
                            out=an, in0=acc[:, 0:128], scalar1=rc)
                        pst = psa.tile([128, 128], F32R, tag="sc", bufs=4)
                        nc.tensor.transpose(pst, an, ident)
                        nc.vector.tensor_copy(stb[b][:, qi, qql, h, :], pst)
                        if h == 1 and op_add is not None:
                            op_add(qql)
                        weave.step()

        # ---- schedule ----
        # phase A: QKV(b0) alone; phase B: att(b0) with QKV(b1) woven in
        # (tensor-dense work fills scalar-exp stalls); phase C: att(b1)
        # with outproj(b0) woven in; tail: outproj(b1).
        ctr = [0]
        _drain(qkv_quanta(0))
        with tc.tile_pool(name="psa0", bufs=1, space="PSUM") as psa0:
            mp = _MultiPacer(112)
            mp.add(qkv_quanta(1), 32)
            for qi in range(NQI):
                attention_unit(0, qi, psa0, mp)
            mp.drain_all()
        with tc.tile_pool(name="psa1", bufs=1, space="PSUM") as psa1:
            mp = _MultiPacer(112)
            for qi in range(NQI):
                mp.add(outproj_quanta(0, qi, psa1, ctr), 16)
            for qi in range(NQI):
                # each qql's outproj joins the weave as soon as its attn^T
                # block completes, so almost nothing is left for the tail
                attention_unit(
                    1, qi, psa1, mp,
                    op_add=lambda qql, qi=qi: mp.add(
                        outproj_quanta(1, qi, psa1, ctr, [qql]), 4))
            mp.drain_all()

    nc.compile()
    return nc


def _get_nc():
    global _cached_nc
    if _cached_nc is None:
        _cached_nc = build_nc()
    return _cached_nc


def make_in_maps(X, Wq, bq, Wk, bk, Wv, bv, Wo, bo):
    X = np.asarray(X, dtype=np.float32)
    scale = np.float32(1.0 / np.sqrt(DH))
    # [128, B, NSC, NDC, SC]: xt[p, b, sc, n, s'] = X[b, sc*SC+s', n*128+p]
    XT = np.ascontiguousarray(
        X.reshape(B, NSC, SC, NDC, 128).transpose(4, 0, 1, 3, 2)
    ).reshape(128, B * NSC, NDC, SC).astype(BF16NP)

    def pmaj(w):  # [D, DHL] -> [128, NDC, DHL] partition-major
        return np.ascontiguousarray(
            w.reshape(NDC, 128, DHL).transpose(1, 0, 2))

    def pmaj_h(w):  # [D, DHL] -> [128, HL, NDC, 128] head-major
        return np.ascontiguousarray(
            w.reshape(NDC, 128, HL, 128).transpose(1, 2, 0, 3))
    mask = np.where(np.arange(128)[None, :] >= np.arange(128)[:, None],
                    np.float32(0.0), np.float32(-1e9)).astype(BF16NP)
    ident = np.eye(128, dtype=np.float32)
    identb = np.eye(128, dtype=np.float32).astype(BF16NP)
    in_maps = []
    for c in range(N_CORES):
        hs = slice(c * DHL, (c + 1) * DHL)
        in_maps.append({
            "xt": XT,
            "wq": pmaj_h(np.asarray(Wq, np.float32)[:, hs] * scale).astype(BF16NP),
            "wk": pmaj(np.asarray(Wk, np.float32)[:, hs]).astype(BF16NP),
            "wv": pmaj(np.asarray(Wv, np.float32)[:, hs]).astype(BF16NP),
            "bqt": np.ascontiguousarray(
                (np.asarray(bq, np.float32)[hs] * scale).reshape(HL, 128).T),
            "wo": np.ascontiguousarray(
                np.asarray(Wo, np.float32)[hs, :].reshape(HL, 128, D)),
            "maskt": mask,
            "ident": ident,
            "identb": identb,
        })
    return in_maps


def kernel(X, Wq, bq, Wk, bk, Wv, bv, Wo, bo, _trace=False):
    nc = _get_nc()
    in_maps = make_in_maps(X, Wq, bq, Wk, bk, Wv, bv, Wo, bo)
    res = run_bass_kernel_spmd(nc, in_maps, list(range(N_CORES)), trace=_trace)
    acc = res.results[0]["out"].astype(np.float64)
    for c in range(1, N_CORES):
        acc += res.results[c]["out"].astype(np.float64)
    # bv commutes through softmax: sum_k w_k (v_k + bv) = (sum_k w_k v_k) + bv,
    # so the V bias contributes bv @ Wo, folded here with bo. bk shifts every
    # score for a given query by the same constant, so it cancels in softmax.
    acc += np.asarray(bo, np.float64) + (
        np.asarray(bv, np.float64) @ np.asarray(Wo, np.float64))
    out = acc.astype(np.float32)
    if _trace:
        return out, res
    return out


# revision 41
# speedup vs baseline: 1.2562x; 1.2562x over previous
"""Trainium2 Bass kernel for multi-head causal self-attention.

Tensor-parallel over 8 NeuronCores: each core owns 2 of the 16 heads.
Per core (SPMD, identical program, different weight shards):
  - QKV projections for its 2 heads (bf16 data, f32 psum accumulate)
  - causal attention for its 2 heads (scores kept transposed [k, q];
    softmax denominator via a ones-column fused into the PV matmul,
    PV streams N=129 in bf16)
  - output projection partial (f32r), deferred and woven into the next
    phase so the tensor engine never starves behind scalar-engine exps
Phase schedule (tensor-heavy QKV woven into scalar-heavy attention):
  QKV(b0) -> att(b0) + QKV(b1) woven -> att(b1) + outproj(b0) woven
  -> outproj(b1) tail.
Host: shards weights, pre-transposes X to bf16, sums the 8 partials,
adds bo (+ bv@Wo; bk drops out of softmax).

Engine split: tensor = all matmuls; scalar = exp only; vector = psum
copy-outs, bias adds, reciprocals, mask multiplies, ones memset.
"""
import numpy as np
from contextlib import ExitStack

import ml_dtypes
import concourse.bass as bass
import concourse.tile as tile
from concourse import bacc, mybir
from concourse.bass_utils import run_bass_kernel_spmd

# Problem shape (hardcoded per contract)
B, S, D = 2, 2048, 2048
H, DH = 16, 128
N_CORES = 8
HL = H // N_CORES          # heads per core = 2
DHL = HL * DH              # 256
SC = 256                   # s-chunk for projections
NSC = S // SC              # 8 chunks per batch
NKB = S // 128             # 16 key blocks per batch
NQI = S // 512             # 4 q-chunks of 512 per batch
NDC = D // 128             # 16 contraction blocks

F32 = mybir.dt.float32
F32R = mybir.dt.float32r
BF16 = mybir.dt.bfloat16
AF = mybir.ActivationFunctionType
BF16NP = ml_dtypes.bfloat16

_cached_nc = None


def _mm(nc, out, lhsT, rhs, start, stop):
    nc.tensor.matmul(out, lhsT, rhs, start=start, stop=stop)


def _drain(gen):
    if gen is not None:
        for _ in gen:
            pass


class _MultiPacer:
    """Round-robin several work generators, paced so the backlog drains
    roughly uniformly over the remaining slots."""

    def __init__(self, total_slots):
        self.total_slots = total_slots
        self.slot = 0
        self.gens = []          # (gen, remaining_estimate)
        self.backlog = 0
        self.credit = 0.0
        self.deferred = []

    def add(self, gen, est):
        self.gens.append(gen)
        self.backlog += est

    def defer(self, b, qi):
        self.deferred.append((b, qi))

    def idle(self):
        return not self.gens

    def step(self):
        self.slot += 1
        remaining_slots = max(1, self.total_slots - self.slot)
        if self.gens:
            self.credit += self.backlog / remaining_slots
        k = int(self.credit)
        for _ in range(k):
            if not self.gens:
                break
            gen = self.gens.pop(0)
            try:
                next(gen)
                self.credit -= 1
                self.backlog = max(0, self.backlog - 1)
                self.gens.append(gen)
            except StopIteration:
                pass

    def drain_all(self):
        for gen in self.gens:
            _drain(gen)
        self.gens = []


class _Pacer:
    """Spread `total` weave quanta over `slots` call sites."""

    def __init__(self, gen, total, slots):
        self.gen = gen
        self.total = total
        self.slots = max(slots, 1)
        self.emitted = 0
        self.slot = 0

    def step(self):
        self.slot += 1
        while (self.gen is not None
               and self.emitted * self.slots < self.slot * self.total):
            try:
                next(self.gen)
                self.emitted += 1
            except StopIteration:
                self.gen = None

    def drain(self):
        _drain(self.gen)
        self.gen = None


def build_nc():
    nc = bacc.Bacc("TRN2", target_bir_lowering=False, debug=False, num_devices=N_CORES)

    # X pre-chunked host-side: xt[p, b, sc, n, s'] = X[b, sc*SC+s', n*128+p]
    xt = nc.dram_tensor("xt", [128, B * NSC, NDC, SC], BF16,
                        kind="ExternalInput").ap()
    # weights pre-arranged host-side to partition-major SBUF layout
    wq = nc.dram_tensor("wq", [128, HL, NDC, 128], BF16,
                        kind="ExternalInput").ap()
    wk = nc.dram_tensor("wk", [128, NDC, DHL], BF16, kind="ExternalInput").ap()
    wv = nc.dram_tensor("wv", [128, NDC, DHL], BF16, kind="ExternalInput").ap()
    bqt_d = nc.dram_tensor("bqt", [128, HL], F32, kind="ExternalInput").ap()
    wo = nc.dram_tensor("wo", [HL, 128, D], F32R, kind="ExternalInput").ap()
    mask_d = nc.dram_tensor("maskt", [128, 128], BF16, kind="ExternalInput").ap()
    ident_d = nc.dram_tensor("ident", [128, 128], F32R, kind="ExternalInput").ap()
    identb_d = nc.dram_tensor("identb", [128, 128], BF16, kind="ExternalInput").ap()
    out = nc.dram_tensor("out", [B, S, D], BF16, kind="ExternalOutput").ap()


    with tile.TileContext(nc) as tc, ExitStack() as ctx:
        pp = ctx.enter_context(tc.tile_pool(name="persist", bufs=1))

        wq_t = pp.tile([128, HL, NDC, 128], BF16)
        wk_t = pp.tile([128, NDC, DHL], BF16)
        wv_t = pp.tile([128, NDC, DHL], BF16)
        wo_t = pp.tile([128, HL, D], F32R)
        bqt = pp.tile([128, HL], F32)
        mask = pp.tile([128, 128], BF16)   # additive: 0 on causal, -1e9 above
        ident = pp.tile([128, 128], F32R)
        identb = pp.tile([128, 128], BF16)

        # h0 slice first: the first Q matmul group gates on 0.5 MB, not 1 MB
        nc.sync.dma_start(out=wq_t[:, 0], in_=wq[:, 0])
        nc.sync.dma_start(out=bqt, in_=bqt_d)
        nc.sync.dma_start(out=wq_t[:, 1], in_=wq[:, 1])

        warm_src = pp.tile([128, 512], BF16, name="warm_src")

        # per-batch persistent activations
        qt = [pp.tile([128, HL, S], BF16, name=f"qt{b}") for b in range(B)]
        kt = [pp.tile([128, HL, S], BF16, name=f"kt{b}") for b in range(B)]
        vcat = [pp.tile([128, NKB, HL, 130], BF16, name=f"vcat{b}")
                for b in range(B)]
        # attn^T blocks awaiting (deferred) output projection
        stb = [pp.tile([128, NQI, 4, HL, 128], F32R, name=f"stb{b}")
               for b in range(B)]

        xp = ctx.enter_context(tc.tile_pool(name="xtp", bufs=3))
        sm = ctx.enter_context(tc.tile_pool(name="smp", bufs=2))
        xpool = ctx.enter_context(tc.tile_pool(name="expool", bufs=2))

        def qkv_quanta(b):
            """Generator: QKV projections for batch b, yielding after each
            16-matmul accumulation group (the weave quantum)."""
            with tc.tile_pool(name=f"psp{b}", bufs=1, space="PSUM") as psp:
                if b == 0:
                    # dummy matmuls fill the startup DMA wait and pre-ramp
                    # the PE clock (full speed needs ~3us continuous work),
                    # so the first real chunks run at 2.4GHz, not 1.2GHz
                    nc.vector.memset(warm_src, 0.5)
                    warm_ps = psp.tile([128, 512], F32, tag="warm", bufs=1)
                    for _ in range(12):
                        _mm(nc, warm_ps, warm_src[:, 0:128], warm_src,
                            True, True)
                nc.vector.memset(vcat[b][:, :, :, 128:129], 1.0)
                def q_group(sc, xt_t, pqk, slot, h):
                    for dc in range(NDC):
                        _mm(nc, pqk[:, slot, :], wq_t[:, h, dc, :],
                            xt_t[:, dc, :], dc == 0, dc == NDC - 1)
                    nc.vector.tensor_scalar_add(
                        out=qt[b][:, h, sc * SC:(sc + 1) * SC],
                        in0=pqk[:, slot, :], scalar1=bqt[:, h:h + 1])

                def k_group(sc, xt_t, pqk, slot, h):
                    for dc in range(NDC):
                        _mm(nc, pqk[:, slot, :],
                            wk_t[:, dc, h * 128:(h + 1) * 128],
                            xt_t[:, dc, :], dc == 0, dc == NDC - 1)
                    nc.vector.tensor_copy(
                        kt[b][:, h, sc * SC:(sc + 1) * SC], pqk[:, slot, :])

                def v_group(sc, xt_t, psv, sb):
                    kb = sc * (SC // 128) + sb
                    pv = psv[:, sb, :]
                    for dc in range(NDC):
                        _mm(nc, pv, xt_t[:, dc, sb * 128:(sb + 1) * 128],
                            wv_t[:, dc, :], dc == 0, dc == NDC - 1)
                    nc.vector.tensor_copy(
                        vcat[b][:, kb, 0, 0:128], pv[:, 0:128])
                    nc.vector.tensor_copy(
                        vcat[b][:, kb, 1, 0:128], pv[:, 128:256])

                for sc in range(NSC):
                    xt_t = xp.tile([128, NDC, SC], BF16, tag="xt")
                    if b == 0 and sc == 0:
                        # two halves so the first matmuls gate on 0.5 MB
                        nc.gpsimd.dma_start(out=xt_t[:, 0:8], in_=xt[:, 0, 0:8])
                        nc.gpsimd.dma_start(out=xt_t[:, 8:16], in_=xt[:, 0, 8:16])
                    else:
                        nc.gpsimd.dma_start(out=xt_t, in_=xt[:, b * NSC + sc])
                    if b == 0 and sc == 0:
                        nc.sync.dma_start(out=wk_t, in_=wk)
                        nc.sync.dma_start(out=wv_t, in_=wv)
                    for h in range(HL):
                        # q and k share one bank: their accumulation groups
                        # are strictly sequential (q closes before k opens)
                        pqk = psp.tile([128, 2, SC], F32, tag="pqk", bufs=1)
                        q_group(sc, xt_t, pqk, 0, h)
                        yield
                        k_group(sc, xt_t, pqk, 1, h)
                        yield
                    psv = psp.tile([128, 2, DHL], F32, tag="pv", bufs=1)
                    for sb in range(SC // 128):
                        v_group(sc, xt_t, psv, sb)
                        yield
                if b == 0:
                    nc.sync.dma_start(out=wo_t[:, 0, :], in_=wo[0])
                    nc.sync.dma_start(out=wo_t[:, 1, :], in_=wo[1])
                    nc.sync.dma_start(out=mask, in_=mask_d)
                    nc.sync.dma_start(out=ident, in_=ident_d)
                    nc.sync.dma_start(out=identb, in_=identb_d)

        def outproj_quanta(b, qi, psa, ctr, qqls=range(4)):
            """Generator: output projection for stb[b][qi], using the po
            tag of the given PSUM pool. Yields per 512-col chunk. Copies
            split 1/3 scalar 2/3 vector: some po-buffer releases bypass
            each queue's head-of-line blocking, without saturating the
            exp-loaded scalar engine."""
            if True:
                for qql in qqls:
                    qq = 4 * qi + qql
                    for dk in range(D // 512):
                        po = psa.tile([128, 512], F32, tag="po", bufs=2)
                        _mm(nc, po, stb[b][:, qi, qql, 0, :],
                            wo_t[:, 0, dk * 512:(dk + 1) * 512], True, False)
                        _mm(nc, po, stb[b][:, qi, qql, 1, :],
                            wo_t[:, 1, dk * 512:(dk + 1) * 512], False, True)
                        ot = sm.tile([128, 512], BF16, tag="ot", bufs=6)
                        ctr[0] += 1
                        if ctr[0] % 3 == 2:
                            nc.scalar.activation(out=ot, in_=po, func=AF.Copy)
                        else:
                            nc.vector.tensor_copy(ot, po)
                        nc.sync.dma_start(
                            out=out[b, qq * 128:(qq + 1) * 128,
                                    dk * 512:(dk + 1) * 512],
                            in_=ot)
                        yield

        def attention_unit(b, qi, psa, weave, op_add=None):
            """One qi-group of causal attention for batch b; calls
            weave.step() at each slot to inject foreign tensor work.
            op_add(qql): called once this qql's attn^T block is complete
            (h==1 done), so its output projection can join the weave."""
            if True:
                for h in range(HL):
                    nkb = 4 * qi + 4
                    pss_t = {}
                    exs = {}

                    def score(kb):
                        dq = max(0, (kb - 4 * qi)) * 128
                        diag = kb >= 4 * qi
                        pss = psa.tile([128, 512], F32, tag="sc", bufs=4)
                        _mm(nc, pss[:, dq:512],
                            kt[b][:, h, kb * 128:(kb + 1) * 128],
                            qt[b][:, h, qi * 512 + dq:(qi + 1) * 512],
                            True, not diag)
                        if diag:
                            # add -1e9 above the diagonal in psum (identity
                            # stationary x additive mask), so exp gives 0 and
                            # no post-exp mask op is needed
                            nc.tensor.matmul(
                                pss[:, dq:dq + 128], identb, mask,
                                start=False, stop=True, skip_group_check=True)
                        pss_t[kb] = pss

                    score(0)
                    for kb in range(nkb):
                        dq = max(0, (kb - 4 * qi)) * 128
                        pss = pss_t.pop(kb)
                        ex = xpool.tile([128, 512], BF16, tag="ex", bufs=18)
                        nc.scalar.activation(
                            out=ex[:, dq:512], in_=pss[:, dq:512],
                            func=AF.Exp)
                        exs[kb] = ex
                        if kb + 1 < nkb:
                            score(kb + 1)
                        weave.step()
                    # qql-outer PV: accumulation groups close sequentially,
                    # so a 2-deep acc ring replaces 4 dedicated banks
                    for qql in range(4):
                        qq = 4 * qi + qql
                        acc = psa.tile([128, 256], F32, tag="acc", bufs=2)
                        for kb in range(qq + 1):
                            _mm(nc, acc[:, 0:129],
                                exs[kb][:, qql * 128:(qql + 1) * 128],
                                vcat[b][:, kb, h, 0:129],
                                kb == 0, kb == qq)
                        rc = sm.tile([128, 1], F32, tag="rc")
                        nc.vector.reciprocal(rc, acc[:, 128:129])
                        an = sm.tile([128, 128], F32R, tag="an")
                        nc.vector.tensor_scalar_mul(
                            out=an, in0=acc[:, 0:128], scalar1=rc)
                        pst = psa.tile([128, 128], F32R, tag="sc", bufs=4)
                        nc.tensor.transpose(pst, an, ident)
                        nc.vector.tensor_copy(stb[b][:, qi, qql, h, :], pst)
                        if h == 1 and op_add is not None:
                            op_add(qql)
                        weave.step()

        # ---- schedule ----
        # phase A: QKV(b0) alone; phase B: att(b0) with QKV(b1) woven in
        # (tensor-dense work fills scalar-exp stalls); phase C: att(b1)
        # with outproj(b0) woven in; tail: outproj(b1).
        ctr = [0]
        _drain(qkv_quanta(0))
        with tc.tile_pool(name="psa0", bufs=1, space="PSUM") as psa0:
            mp = _MultiPacer(112)
            mp.add(qkv_quanta(1), 48)
            for qi in range(NQI):
                attention_unit(0, qi, psa0, mp)
            mp.drain_all()
        with tc.tile_pool(name="psa1", bufs=1, space="PSUM") as psa1:
            mp = _MultiPacer(112)
            for qi in range(NQI):
                mp.add(outproj_quanta(0, qi, psa1, ctr), 16)
            for qi in range(NQI):
                # each qql's outproj joins the weave as soon as its attn^T
                # block completes, so almost nothing is left for the tail
                attention_unit(
                    1, qi, psa1, mp,
                    op_add=lambda qql, qi=qi: mp.add(
                        outproj_quanta(1, qi, psa1, ctr, [qql]), 4))
            mp.drain_all()

    nc.compile()
    return nc


def _get_nc():
    global _cached_nc
    if _cached_nc is None:
        _cached_nc = build_nc()
    return _cached_nc


def make_in_maps(X, Wq, bq, Wk, bk, Wv, bv, Wo, bo):
    X = np.asarray(X, dtype=np.float32)
    scale = np.float32(1.0 / np.sqrt(DH))
    # [128, B, NSC, NDC, SC]: xt[p, b, sc, n, s'] = X[b, sc*SC+s', n*128+p]
    XT = np.ascontiguousarray(
        X.reshape(B, NSC, SC, NDC, 128).transpose(4, 0, 1, 3, 2)
    ).reshape(128, B * NSC, NDC, SC).astype(BF16NP)

    def pmaj(w):  # [D, DHL] -> [128, NDC, DHL] partition-major
        return np.ascontiguousarray(
            w.reshape(NDC, 128, DHL).transpose(1, 0, 2))

    def pmaj_h(w):  # [D, DHL] -> [128, HL, NDC, 128] head-major
        return np.ascontiguousarray(
            w.reshape(NDC, 128, HL, 128).transpose(1, 2, 0, 3))
    mask = np.where(np.arange(128)[None, :] >= np.arange(128)[:, None],
                    np.float32(0.0), np.float32(-1e9)).astype(BF16NP)
    ident = np.eye(128, dtype=np.float32)
    identb = np.eye(128, dtype=np.float32).astype(BF16NP)
    in_maps = []
    for c in range(N_CORES):
        hs = slice(c * DHL, (c + 1) * DHL)
        in_maps.append({
            "xt": XT,
            "wq": pmaj_h(np.asarray(Wq, np.float32)[:, hs] * scale).astype(BF16NP),
            "wk": pmaj(np.asarray(Wk, np.float32)[:, hs]).astype(BF16NP),
            "wv": pmaj(np.asarray(Wv, np.float32)[:, hs]).astype(BF16NP),
            "bqt": np.ascontiguousarray(
                (np.asarray(bq, np.float32)[hs] * scale).reshape(HL, 128).T),
            "wo": np.ascontiguousarray(
                np.asarray(Wo, np.float32)[hs, :].reshape(HL, 128, D)),
            "maskt": mask,
            "ident": ident,
            "identb": identb,
        })
    return in_maps


def kernel(X, Wq, bq, Wk, bk, Wv, bv, Wo, bo, _trace=False):
    nc = _get_nc()
    in_maps = make_in_maps(X, Wq, bq, Wk, bk, Wv, bv, Wo, bo)
    res = run_bass_kernel_spmd(nc, in_maps, list(range(N_CORES)), trace=_trace)
    acc = res.results[0]["out"].astype(np.float64)
    for c in range(1, N_CORES):
        acc += res.results[c]["out"].astype(np.float64)
    # bv commutes through softmax: sum_k w_k (v_k + bv) = (sum_k w_k v_k) + bv,
    # so the V bias contributes bv @ Wo, folded here with bo. bk shifts every
    # score for a given query by the same constant, so it cancels in softmax.
    acc += np.asarray(bo, np.float64) + (
        np.asarray(bv, np.float64) @ np.asarray(Wo, np.float64))
    out = acc.astype(np.float32)
    if _trace:
        return out, res
    return out


# revision 42
# speedup vs baseline: 1.2614x; 1.0041x over previous
"""Trainium2 Bass kernel for multi-head causal self-attention.

Tensor-parallel over 8 NeuronCores: each core owns 2 of the 16 heads.
Per core (SPMD, identical program, different weight shards):
  - QKV projections for its 2 heads (bf16 data, f32 psum accumulate)
  - causal attention for its 2 heads (scores kept transposed [k, q];
    softmax denominator via a ones-column fused into the PV matmul,
    PV streams N=129 in bf16)
  - output projection partial (f32r), deferred and woven into the next
    phase so the tensor engine never starves behind scalar-engine exps
Phase schedule (tensor-heavy QKV woven into scalar-heavy attention):
  QKV(b0) -> att(b0) + QKV(b1) woven -> att(b1) + outproj(b0) woven
  -> outproj(b1) tail.
Host: shards weights, pre-transposes X to bf16, sums the 8 partials,
adds bo (+ bv@Wo; bk drops out of softmax).

Engine split: tensor = all matmuls; scalar = exp only; vector = psum
copy-outs, bias adds, reciprocals, mask multiplies, ones memset.
"""
import numpy as np
from contextlib import ExitStack

import ml_dtypes
import concourse.bass as bass
import concourse.tile as tile
from concourse import bacc, mybir
from concourse.bass_utils import run_bass_kernel_spmd

# Problem shape (hardcoded per contract)
B, S, D = 2, 2048, 2048
H, DH = 16, 128
N_CORES = 8
HL = H // N_CORES          # heads per core = 2
DHL = HL * DH              # 256
SC = 256                   # s-chunk for projections
NSC = S // SC              # 8 chunks per batch
NKB = S // 128             # 16 key blocks per batch
NQI = S // 512             # 4 q-chunks of 512 per batch
NDC = D // 128             # 16 contraction blocks

F32 = mybir.dt.float32
F32R = mybir.dt.float32r
BF16 = mybir.dt.bfloat16
AF = mybir.ActivationFunctionType
BF16NP = ml_dtypes.bfloat16

_cached_nc = None


def _mm(nc, out, lhsT, rhs, start, stop):
    nc.tensor.matmul(out, lhsT, rhs, start=start, stop=stop)


def _drain(gen):
    if gen is not None:
        for _ in gen:
            pass


class _MultiPacer:
    """Round-robin several work generators, paced so the backlog drains
    roughly uniformly over the remaining slots."""

    def __init__(self, total_slots):
        self.total_slots = total_slots
        self.slot = 0
        self.gens = []          # (gen, remaining_estimate)
        self.backlog = 0
        self.credit = 0.0
        self.deferred = []

    def add(self, gen, est):
        self.gens.append(gen)
        self.backlog += est

    def defer(self, b, qi):
        self.deferred.append((b, qi))

    def idle(self):
        return not self.gens

    def step(self):
        self.slot += 1
        remaining_slots = max(1, self.total_slots - self.slot)
        if self.gens:
            self.credit += self.backlog / remaining_slots
        k = int(self.credit)
        for _ in range(k):
            if not self.gens:
                break
            gen = self.gens.pop(0)
            try:
                next(gen)
                self.credit -= 1
                self.backlog = max(0, self.backlog - 1)
                self.gens.append(gen)
            except StopIteration:
                pass

    def drain_all(self):
        for gen in self.gens:
            _drain(gen)
        self.gens = []


class _Pacer:
    """Spread `total` weave quanta over `slots` call sites."""

    def __init__(self, gen, total, slots):
        self.gen = gen
        self.total = total
        self.slots = max(slots, 1)
        self.emitted = 0
        self.slot = 0

    def step(self):
        self.slot += 1
        while (self.gen is not None
               and self.emitted * self.slots < self.slot * self.total):
            try:
                next(self.gen)
                self.emitted += 1
            except StopIteration:
                self.gen = None

    def drain(self):
        _drain(self.gen)
        self.gen = None


def build_nc():
    nc = bacc.Bacc("TRN2", target_bir_lowering=False, debug=False, num_devices=N_CORES)

    # X pre-chunked host-side: xt[p, b, sc, n, s'] = X[b, sc*SC+s', n*128+p]
    xt = nc.dram_tensor("xt", [128, B * NSC, NDC, SC], BF16,
                        kind="ExternalInput").ap()
    # weights pre-arranged host-side to partition-major SBUF layout
    wq = nc.dram_tensor("wq", [128, HL, NDC, 128], BF16,
                        kind="ExternalInput").ap()
    wk = nc.dram_tensor("wk", [128, NDC, DHL], BF16, kind="ExternalInput").ap()
    wv = nc.dram_tensor("wv", [128, NDC, DHL], BF16, kind="ExternalInput").ap()
    bqt_d = nc.dram_tensor("bqt", [128, HL], F32, kind="ExternalInput").ap()
    wo = nc.dram_tensor("wo", [HL, 128, D], F32R, kind="ExternalInput").ap()
    mask_d = nc.dram_tensor("maskt", [128, 128], BF16, kind="ExternalInput").ap()
    ident_d = nc.dram_tensor("ident", [128, 128], F32R, kind="ExternalInput").ap()
    identb_d = nc.dram_tensor("identb", [128, 128], BF16, kind="ExternalInput").ap()
    out = nc.dram_tensor("out", [B, S, D], BF16, kind="ExternalOutput").ap()


    with tile.TileContext(nc) as tc, ExitStack() as ctx:
        pp = ctx.enter_context(tc.tile_pool(name="persist", bufs=1))

        wq_t = pp.tile([128, HL, NDC, 128], BF16)
        wk_t = pp.tile([128, NDC, DHL], BF16)
        wv_t = pp.tile([128, NDC, DHL], BF16)
        wo_t = pp.tile([128, HL, D], F32R)
        bqt = pp.tile([128, HL], F32)
        mask = pp.tile([128, 128], BF16)   # additive: 0 on causal, -1e9 above
        ident = pp.tile([128, 128], F32R)
        identb = pp.tile([128, 128], BF16)

        # h0 slice first: the first Q matmul group gates on 0.5 MB, not 1 MB
        nc.sync.dma_start(out=wq_t[:, 0], in_=wq[:, 0])
        nc.sync.dma_start(out=bqt, in_=bqt_d)
        nc.sync.dma_start(out=wq_t[:, 1], in_=wq[:, 1])

        warm_src = pp.tile([128, 512], BF16, name="warm_src")

        # per-batch persistent activations
        qt = [pp.tile([128, HL, S], BF16, name=f"qt{b}") for b in range(B)]
        kt = [pp.tile([128, HL, S], BF16, name=f"kt{b}") for b in range(B)]
        vcat = [pp.tile([128, NKB, HL, 130], BF16, name=f"vcat{b}")
                for b in range(B)]
        # attn^T blocks awaiting (deferred) output projection
        stb = [pp.tile([128, NQI, 4, HL, 128], F32R, name=f"stb{b}")
               for b in range(B)]

        xp = ctx.enter_context(tc.tile_pool(name="xtp", bufs=3))
        sm = ctx.enter_context(tc.tile_pool(name="smp", bufs=2))
        xpool = ctx.enter_context(tc.tile_pool(name="expool", bufs=2))

        def qkv_quanta(b):
            """Generator: QKV projections for batch b, yielding after each
            16-matmul accumulation group (the weave quantum)."""
            with tc.tile_pool(name=f"psp{b}", bufs=1, space="PSUM") as psp:
                if b == 0:
                    # dummy matmuls fill the startup DMA wait and pre-ramp
                    # the PE clock (full speed needs ~3us continuous work),
                    # so the first real chunks run at 2.4GHz, not 1.2GHz
                    nc.vector.memset(warm_src, 0.5)
                    warm_ps = psp.tile([128, 512], F32, tag="warm", bufs=1)
                    for _ in range(12):
                        _mm(nc, warm_ps, warm_src[:, 0:128], warm_src,
                            True, True)
                nc.vector.memset(vcat[b][:, :, :, 128:129], 1.0)
                def q_group(sc, xt_t, pqk, slot, h):
                    for dc in range(NDC):
                        _mm(nc, pqk[:, slot, :], wq_t[:, h, dc, :],
                            xt_t[:, dc, :], dc == 0, dc == NDC - 1)
                    nc.vector.tensor_scalar_add(
                        out=qt[b][:, h, sc * SC:(sc + 1) * SC],
                        in0=pqk[:, slot, :], scalar1=bqt[:, h:h + 1])

                def k_group(sc, xt_t, pqk, slot, h):
                    for dc in range(NDC):
                        _mm(nc, pqk[:, slot, :],
                            wk_t[:, dc, h * 128:(h + 1) * 128],
                            xt_t[:, dc, :], dc == 0, dc == NDC - 1)
                    nc.vector.tensor_copy(
                        kt[b][:, h, sc * SC:(sc + 1) * SC], pqk[:, slot, :])

                def v_group(sc, xt_t, psv, sb):
                    kb = sc * (SC // 128) + sb
                    pv = psv[:, sb, :]
                    for dc in range(NDC):
                        _mm(nc, pv, xt_t[:, dc, sb * 128:(sb + 1) * 128],
                            wv_t[:, dc, :], dc == 0, dc == NDC - 1)
                    nc.vector.tensor_copy(
                        vcat[b][:, kb, 0, 0:128], pv[:, 0:128])
                    nc.vector.tensor_copy(
                        vcat[b][:, kb, 1, 0:128], pv[:, 128:256])

                for sc in range(NSC):
                    xt_t = xp.tile([128, NDC, SC], BF16, tag="xt")
                    if b == 0 and sc == 0:
                        # two halves so the first matmuls gate on 0.5 MB
                        nc.gpsimd.dma_start(out=xt_t[:, 0:8], in_=xt[:, 0, 0:8])
                        nc.gpsimd.dma_start(out=xt_t[:, 8:16], in_=xt[:, 0, 8:16])
                    else:
                        nc.gpsimd.dma_start(out=xt_t, in_=xt[:, b * NSC + sc])
                    if b == 0 and sc == 0:
                        nc.sync.dma_start(out=wk_t, in_=wk)
                        nc.sync.dma_start(out=wv_t, in_=wv)
                    for h in range(HL):
                        # q and k share one bank: their accumulation groups
                        # are strictly sequential (q closes before k opens)
                        pqk = psp.tile([128, 2, SC], F32, tag="pqk", bufs=1)
                        q_group(sc, xt_t, pqk, 0, h)
                        yield
                        k_group(sc, xt_t, pqk, 1, h)
                        yield
                    psv = psp.tile([128, 2, DHL], F32, tag="pv", bufs=1)
                    for sb in range(SC // 128):
                        v_group(sc, xt_t, psv, sb)
                        yield
                if b == 0:
                    nc.sync.dma_start(out=wo_t[:, 0, :], in_=wo[0])
                    nc.sync.dma_start(out=wo_t[:, 1, :], in_=wo[1])
                    nc.sync.dma_start(out=mask, in_=mask_d)
                    nc.sync.dma_start(out=ident, in_=ident_d)
                    nc.sync.dma_start(out=identb, in_=identb_d)

        def outproj_quanta(b, qi, psa, ctr, qqls=range(4)):
            """Generator: output projection for stb[b][qi], using the po
            tag of the given PSUM pool. Yields per 512-col chunk. Copies
            split 1/3 scalar 2/3 vector: some po-buffer releases bypass
            each queue's head-of-line blocking, without saturating the
            exp-loaded scalar engine."""
            if True:
                for qql in qqls:
                    qq = 4 * qi + qql
                    for dk in range(D // 512):
                        po = psa.tile([128, 512], F32, tag="po", bufs=3)
                        _mm(nc, po, stb[b][:, qi, qql, 0, :],
                            wo_t[:, 0, dk * 512:(dk + 1) * 512], True, False)
                        _mm(nc, po, stb[b][:, qi, qql, 1, :],
                            wo_t[:, 1, dk * 512:(dk + 1) * 512], False, True)
                        ot = sm.tile([128, 512], BF16, tag="ot", bufs=6)
                        ctr[0] += 1
                        if ctr[0] % 3 == 2:
                            nc.scalar.activation(out=ot, in_=po, func=AF.Copy)
                        else:
                            nc.vector.tensor_copy(ot, po)
                        nc.sync.dma_start(
                            out=out[b, qq * 128:(qq + 1) * 128,
                                    dk * 512:(dk + 1) * 512],
                            in_=ot)
                        yield

        def attention_unit(b, qi, psa, weave, op_add=None):
            """One qi-group of causal attention for batch b; calls
            weave.step() at each slot to inject foreign tensor work.
            op_add(qql): called once this qql's attn^T block is complete
            (h==1 done), so its output projection can join the weave."""
            if True:
                for h in range(HL):
                    nkb = 4 * qi + 4
                    pss_t = {}
                    exs = {}

                    def score(kb):
                        dq = max(0, (kb - 4 * qi)) * 128
                        diag = kb >= 4 * qi
                        pss = psa.tile([128, 512], F32, tag="sc", bufs=3)
                        _mm(nc, pss[:, dq:512],
                            kt[b][:, h, kb * 128:(kb + 1) * 128],
                            qt[b][:, h, qi * 512 + dq:(qi + 1) * 512],
                            True, not diag)
                        if diag:
                            # add -1e9 above the diagonal in psum (identity
                            # stationary x additive mask), so exp gives 0 and
                            # no post-exp mask op is needed
                            nc.tensor.matmul(
                                pss[:, dq:dq + 128], identb, mask,
                                start=False, stop=True, skip_group_check=True)
                        pss_t[kb] = pss

                    score(0)
                    for kb in range(nkb):
                        dq = max(0, (kb - 4 * qi)) * 128
                        pss = pss_t.pop(kb)
                        ex = xpool.tile([128, 512], BF16, tag="ex", bufs=18)
                        nc.scalar.activation(
                            out=ex[:, dq:512], in_=pss[:, dq:512],
                            func=AF.Exp)
                        exs[kb] = ex
                        if kb + 1 < nkb:
                            score(kb + 1)
                        weave.step()
                    # qql-outer PV: accumulation groups close sequentially,
                    # so a 2-deep acc ring replaces 4 dedicated banks
                    for qql in range(4):
                        qq = 4 * qi + qql
                        acc = psa.tile([128, 256], F32, tag="acc", bufs=2)
                        for kb in range(qq + 1):
                            _mm(nc, acc[:, 0:129],
                                exs[kb][:, qql * 128:(qql + 1) * 128],
                                vcat[b][:, kb, h, 0:129],
                                kb == 0, kb == qq)
                        rc = sm.tile([128, 1], F32, tag="rc")
                        nc.vector.reciprocal(rc, acc[:, 128:129])
                        an = sm.tile([128, 128], F32R, tag="an")
                        nc.vector.tensor_scalar_mul(
                            out=an, in0=acc[:, 0:128], scalar1=rc)
                        pst = psa.tile([128, 128], F32R, tag="sc", bufs=3)
                        nc.tensor.transpose(pst, an, ident)
                        nc.vector.tensor_copy(stb[b][:, qi, qql, h, :], pst)
                        if h == 1 and op_add is not None:
                            op_add(qql)
                        weave.step()

        # ---- schedule ----
        # phase A: QKV(b0) alone; phase B: att(b0) with QKV(b1) woven in
        # (tensor-dense work fills scalar-exp stalls); phase C: att(b1)
        # with outproj(b0) woven in; tail: outproj(b1).
        ctr = [0]
        _drain(qkv_quanta(0))
        with tc.tile_pool(name="psa0", bufs=1, space="PSUM") as psa0:
            mp = _MultiPacer(112)
            mp.add(qkv_quanta(1), 48)
            for qi in range(NQI):
                attention_unit(0, qi, psa0, mp)
            mp.drain_all()
        with tc.tile_pool(name="psa1", bufs=1, space="PSUM") as psa1:
            mp = _MultiPacer(112)
            for qi in range(NQI):
                mp.add(outproj_quanta(0, qi, psa1, ctr), 16)
            for qi in range(NQI):
                # each qql's outproj joins the weave as soon as its attn^T
                # block completes, so almost nothing is left for the tail
                attention_unit(
                    1, qi, psa1, mp,
                    op_add=lambda qql, qi=qi: mp.add(
                        outproj_quanta(1, qi, psa1, ctr, [qql]), 4))
            mp.drain_all()

    nc.compile()
    return nc


def _get_nc():
    global _cached_nc
    if _cached_nc is None:
        _cached_nc = build_nc()
    return _cached_nc


def make_in_maps(X, Wq, bq, Wk, bk, Wv, bv, Wo, bo):
    X = np.asarray(X, dtype=np.float32)
    scale = np.float32(1.0 / np.sqrt(DH))
    # [128, B, NSC, NDC, SC]: xt[p, b, sc, n, s'] = X[b, sc*SC+s', n*128+p]
    XT = np.ascontiguousarray(
        X.reshape(B, NSC, SC, NDC, 128).transpose(4, 0, 1, 3, 2)
    ).reshape(128, B * NSC, NDC, SC).astype(BF16NP)

    def pmaj(w):  # [D, DHL] -> [128, NDC, DHL] partition-major
        return np.ascontiguousarray(
            w.reshape(NDC, 128, DHL).transpose(1, 0, 2))

    def pmaj_h(w):  # [D, DHL] -> [128, HL, NDC, 128] head-major
        return np.ascontiguousarray(
            w.reshape(NDC, 128, HL, 128).transpose(1, 2, 0, 3))
    mask = np.where(np.arange(128)[None, :] >= np.arange(128)[:, None],
                    np.float32(0.0), np.float32(-1e9)).astype(BF16NP)
    ident = np.eye(128, dtype=np.float32)
    identb = np.eye(128, dtype=np.float32).astype(BF16NP)
    in_maps = []
    for c in range(N_CORES):
        hs = slice(c * DHL, (c + 1) * DHL)
        in_maps.append({
            "xt": XT,
            "wq": pmaj_h(np.asarray(Wq, np.float32)[:, hs] * scale).astype(BF16NP),
            "wk": pmaj(np.asarray(Wk, np.float32)[:, hs]).astype(BF16NP),
            "wv": pmaj(np.asarray(Wv, np.float32)[:, hs]).astype(BF16NP),
            "bqt": np.ascontiguousarray(
                (np.asarray(bq, np.float32)[hs] * scale).reshape(HL, 128).T),
            "wo": np.ascontiguousarray(
                np.asarray(Wo, np.float32)[hs, :].reshape(HL, 128, D)),
            "maskt": mask,
            "ident": ident,
            "identb": identb,
        })
    return in_maps


def kernel(X, Wq, bq, Wk, bk, Wv, bv, Wo, bo, _trace=False):
    nc = _get_nc()
    in_maps = make_in_maps(X, Wq, bq, Wk, bk, Wv, bv, Wo, bo)
    res = run_bass_kernel_spmd(nc, in_maps, list(range(N_CORES)), trace=_trace)
    acc = res.results[0]["out"].astype(np.float64)
    for c in range(1, N_CORES):
        acc += res.results[c]["out"].astype(np.float64)
    # bv commutes through softmax: sum_k w_k (v_k + bv) = (sum_k w_k v_k) + bv,
    # so the V bias contributes bv @ Wo, folded here with bo. bk shifts every
    # score for a given query by the same constant, so it cancels in softmax.
    acc += np.asarray(bo, np.float64) + (
        np.asarray(bv, np.float64) @ np.asarray(Wo, np.float64))
    out = acc.astype(np.float32)
    if _trace:
        return out, res
    return out
